# revision 1
# baseline (speedup 1.0000x reference)
"""AlphaFold-style gated MSA-row attention on 8 Trainium2 NeuronCores.

Shapes: q_data/kv_data [1,128,256,256], bias [1,128,8,256,256],
nonbatched_bias [1,8,256,256]; heads=8, c=32, out=256.

Strategy: pure data-parallel over b2 (128 rows -> 16 rows/core).
Per row, activations stay in "transposed" layout so softmax never needs an
on-chip transpose:
  qT/kT/gT [hc, lq]  (hc = 8*32 = 256, two 128-partition chunks)
  v        [lk, hc]
  L^T      [lk, lq]  per head = K Q^T via K=32 row-tiled matmuls (4 heads
                     run concurrently in the 32x128-tiled PE array)
  + bias^T/nb^T via 32x32 shifted-identity diag tiles: all 16 PE cells add
    partition-rotated bias content concurrently (host pre-rotates layouts)
  E^T      = exp(L^T)  (ACT, PSUM->SBUF, batched 4 heads per instr)
  S_bc     = (2*ones)^T E^T  -> broadcast of 2*rowsum over 32 partitions/head
  waT      = v^T E^T (head-packed via PSUM col tiling)
  m^T      = waT * (1 + tanh((g+bg)/2)) * recip(S_bc)
  out      = m^T^T Wo^T + bo

All matmuls run bf16 inputs (full PE speed, psum accumulates f32); all large
HBM traffic (bias, nb, q, kv, weights) is bf16 -> ~26 MB/core vs 45 f32.
Host side only re-lays-out / dtype-converts inputs - no arithmetic.
"""

import numpy as np

B1, B2, LQ, LK = 1, 128, 256, 256
QD = KVD = 256
H, C = 8, 32
HC = H * C          # 256
OD = 256
NCORES = 8
RPC = B2 // NCORES  # 16 rows per core

_CACHE = {}
_NO_ADDS = False
_NO_WEAVE = False
_NB_DVE = True
_ADD_SRCS = (0, 1)  # which of (bias, nb) to add


def _build_nc():
    import concourse.bass as bass
    import concourse.bacc as bacc
    import concourse.mybir as mybir
    from concourse.tile import TileContext
    from concourse.bass import ts
    from concourse.tile_rust import add_dep_helper

    f32 = mybir.dt.float32
    bf16 = mybir.dt.bfloat16
    AF = mybir.ActivationFunctionType
    ALU = mybir.AluOpType

    nc = bacc.Bacc()

    # ---- DRAM parameters (per-core shard shapes, mostly bf16) ----
    qdT_d = nc.declare_dram_parameter("qdT", [RPC // 2, 2, 128, 2, LQ], bf16, isOutput=False)
    kvdT_d = nc.declare_dram_parameter("kvdT", [RPC // 2, 2, 128, 2, LK], bf16, isOutput=False)
    # bias: [row, rr?] packed per PAIR: [sp, 2(rr), 2(c2), 128, H, LQ]
    biasT_d = nc.declare_dram_parameter("biasT", [RPC // 2, 2, 2, 128, H, LQ], bf16, isOutput=False)
    nbT_d = nc.declare_dram_parameter("nbT", [2, 128, H, LQ], bf16, isOutput=False)
    ident_d = nc.declare_dram_parameter("ident", [128, 128], bf16, isOutput=False)
    wqT_d = nc.declare_dram_parameter("wqT", [QD, HC], bf16, isOutput=False)
    wkT_d = nc.declare_dram_parameter("wkT", [KVD, HC], bf16, isOutput=False)
    wvT_d = nc.declare_dram_parameter("wvT", [KVD, HC], bf16, isOutput=False)
    wgT_d = nc.declare_dram_parameter("wgT", [QD, HC], bf16, isOutput=False)
    woT_d = nc.declare_dram_parameter("woT", [HC, OD], bf16, isOutput=False)
    bg_d = nc.declare_dram_parameter("bg", [HC], f32, isOutput=False)
    bob_d = nc.declare_dram_parameter("bo_bcast", [128, 2, OD], f32, isOutput=False)
    two32_d = nc.declare_dram_parameter("two32", [128, 32], bf16, isOutput=False)
    out_d = nc.declare_dram_parameter("out", [RPC, LQ, OD], f32, isOutput=True)

    def chain(mms):
        for a, b in zip(mms, mms[1:]):
            add_dep_helper(b.ins, a.ins, sync=False, reason="psum bank group order")

    with TileContext(nc) as tc:
        with tc.tile_pool(name="const", bufs=1) as constp, \
             tc.tile_pool(name="io", bufs=2) as iop, \
             tc.tile_pool(name="act", bufs=2) as actp, \
             tc.tile_pool(name="ps", bufs=1, space="PSUM") as psp:

            # ---- one-time loads ----
            wq = constp.tile([128, 2, HC], bf16, name="wq")
            wk = constp.tile([128, 2, HC], bf16, name="wk")
            wv = constp.tile([128, 2, HC], bf16, name="wv")
            wg = constp.tile([128, 2, HC], bf16, name="wg")
            wo = constp.tile([128, 2, OD], bf16, name="wo")
            for t, d in ((wq, wqT_d), (wk, wkT_d), (wv, wvT_d), (wg, wgT_d)):
                nc.sync.dma_start(out=t, in_=d.rearrange("(c p) h -> p c h", p=128))
            nbt = constp.tile([128, 2, 4, 2, LQ], bf16, name="nbt")
            enb = constp.tile([128, 2, 4, 2, LQ], bf16, name="enb")
            ident = constp.tile([128, 128], bf16, name="ident")
            two32 = constp.tile([128, 32], bf16, name="two32")
            bob = constp.tile([128, 2, OD], f32, name="bob")
            bg_sb = constp.tile([128, 2], f32, name="bg_sb")
            bgh = constp.tile([128, 2], f32, name="bgh")

            def load_consts():
                nc.sync.dma_start(
                    out=wo, in_=woT_d.rearrange("(c p) h -> p c h", p=128))
                nc.sync.dma_start(
                    out=nbt, in_=nbT_d.rearrange("c p (k h) l -> p c k h l", k=4))
                nc.sync.dma_start(out=ident, in_=ident_d[:, :])
                nc.scalar.activation(enb, nbt, AF.Exp)
                nc.sync.dma_start(out=two32, in_=two32_d[:, :])
                nc.sync.dma_start(out=bob, in_=bob_d[:, :, :])
                nc.sync.dma_start(out=bg_sb, in_=bg_d.rearrange("(c p) -> p c", p=128))
                nc.scalar.mul(bgh, bg_sb, 0.5)

            qscale = float(C) ** -0.5

            def emit_pair(sp):
                # ---- input DMAs (pair) ----
                qdt = iop.tile([128, 2, 2, LQ], bf16, tag="qdt", name="qdt")
                kvdt = iop.tile([128, 2, 2, LK], bf16, tag="kvdt", name="kvdt")
                nc.sync.dma_start(out=qdt, in_=qdT_d[sp].rearrange("c p r l -> p c r l"))
                nc.sync.dma_start(out=kvdt, in_=kvdT_d[sp].rearrange("c p r l -> p c r l"))
                # bias for both rows of the pair in ONE dispatch
                bias_sb = iop.tile([128, 2, 2, H, LQ], bf16, tag="bias", bufs=3, name="bias_sb")
                nc.sync.dma_start(
                    out=bias_sb, in_=biasT_d[sp].rearrange("r c p h l -> p r c h l"))
                # ---- paired projections q/k/g (N=512 over 2 rows) ----
                pq_j = [psp.tile([128, 2, LQ], f32, tag="proj", bufs=2, name=f"pq{j}") for j in range(2)]
                pk_j = [psp.tile([128, 2, LK], f32, tag="proj", bufs=2, name=f"pk{j}") for j in range(2)]
                pg_j = [psp.tile([128, 2, LQ], f32, tag="proj", bufs=2, name=f"pg{j}") for j in range(2)]
                for pX, w, rhs_t in ((pq_j, wq, qdt), (pk_j, wk, kvdt), (pg_j, wg, qdt)):
                    for j in range(2):
                        mms = []
                        for c2 in range(2):
                            mms.append(nc.tensor.matmul(
                                pX[j][:, :, :], w[:, c2, ts(j, 128)], rhs_t[:, c2, :, :],
                                start=(c2 == 0), stop=(c2 == 1)))
                        chain(mms)
                if sp == 0:
                    load_consts()

                sq = actp.tile([128, 2, 2, LQ], bf16, tag="sq", name="sq")
                sk = actp.tile([128, 2, 2, LK], bf16, tag="sk", name="sk")
                tv = actp.tile([128, 2, 2, LQ], f32, tag="tv", name="tv")
                for j in range(2):
                    nc.vector.tensor_scalar_mul(sq[:, j, :, :], pq_j[j], qscale)
                    nc.vector.tensor_copy(sk[:, j, :, :], pk_j[j])
                    nc.scalar.activation(tv[:, j, :, :], pg_j[j], AF.Tanh,
                                         bias=bgh[:, ts(j, 1)], scale=0.5)

                sv_r = []
                for rr in range(2):
                    # ---- v projection (per row) ----
                    pv = psp.tile([128, 2, HC], f32, tag="proj", bufs=2, name="pv")
                    mm_v = []
                    for j in range(2):
                        for c2 in range(2):
                            mm_v.append(nc.tensor.matmul(
                                pv[:, j, :], kvdt[:, c2, rr, ts(j, 128)], wv[:, c2, :],
                                start=(c2 == 0), stop=(c2 == 1)))
                    chain(mm_v)
                    sv = actp.tile([128, 2, HC], bf16, tag="sv", bufs=4, name="sv")
                    nc.vector.tensor_copy(sv, pv)
                    sv_r.append(sv)
                return dict(sp=sp, bias_sb=bias_sb, sq=sq, sk=sk, tv=tv, sv_r=sv_r)

            # Software pipeline: the sbc/av ("B") PE work of row r-1 is emitted
            # BETWEEN the logits/exp ("L") chunks of row r, so the PE never
            # idles waiting for exp (pL is single-buffered).
            pending = []

            def drain_one():
                if pending:
                    pending.pop(0)()

            def flush_pending():
                while pending:
                    pending.pop(0)()

            def emit_attn_L(st, rr, c2, et, et2):
                sq, sk, bias_sb = st["sq"], st["sk"], st["bias_sb"]
                pL = psp.tile([128, 4, 2, LQ], f32, tag="L", bufs=1, name="pL")
                # start=True resets the WHOLE psum bank -> exactly one start
                # per bank (head h, half=0); head h+4 (same bank, other column
                # half) accumulates onto the zeroed bank.
                grp = [[] for _ in range(4)]
                for h in range(H):
                    k, half = h % 4, h // 4
                    grp[k].append(nc.tensor.matmul(
                        pL[:, k, half, :],
                        sk[ts(k, 32), h // 4, rr, ts(c2, 128)],
                        sq[ts(k, 32), h // 4, rr, :],
                        start=(half == 0), stop=False, skip_group_check=True,
                        tile_position=(32 * k, 0)))
                # bias + nb adds: wide full-identity matmuls per bank
                for k in range(4):
                    grp[k].append(nc.tensor.matmul(
                        pL[:, k, :, :], ident,
                        bias_sb[:, rr, c2, ts(k, 2), :],
                        start=False, stop=_NB_DVE, skip_group_check=True))
                    if not _NB_DVE:
                        grp[k].append(nc.tensor.matmul(
                            pL[:, k, :, :], ident,
                            nbt[:, c2, k, :, :],
                            start=False, stop=True, skip_group_check=True))
                for k in range(4):
                    chain(grp[k])
                for bh in range(2):
                    nc.scalar.activation(
                        et[:, c2, ts(bh, 2), :, :], pL[:, ts(bh, 2), :, :],
                        AF.Exp)
                    if _NB_DVE:
                        # nb applied multiplicatively: E2 = E * exp(nb^T) via
                        # plain InstTensorTensor (2x_1p-capable), not in-place
                        nc.vector.tensor_mul(
                            et2[:, c2, ts(bh, 2), :, :],
                            et[:, c2, ts(bh, 2), :, :],
                            enb[:, c2, ts(bh, 2), :, :])

            def emit_attn_B1(st, rr, et, sbc, av):
                # sbc/av: full contraction, v4 emission order (g, c2, hh)
                sv = st["sv_r"][rr]
                mm_s, mm_a = [], []
                for g in range(2):
                    for c2 in range(2):
                        for hh in range(4):
                            h = 4 * g + hh
                            eth = et[:, c2, h % 4, h // 4, :]
                            mm_s.append(nc.tensor.matmul(
                                sbc[ts(hh, 32), g, :], two32, eth,
                                start=(c2 == 0), stop=(c2 == 1),
                                tile_position=(0, 32 * hh), skip_group_check=True))
                            mm_a.append(nc.tensor.matmul(
                                av[ts(hh, 32), g, :], sv[:, c2, ts(h, 32)], eth,
                                start=(c2 == 0), stop=(c2 == 1),
                                tile_position=(0, 32 * hh), skip_group_check=True))
                chain(mm_s)
                chain(mm_a)
                return None

            def emit_attn_B2(st, rr, s, sbc, av):
                tv = st["tv"]
                rs = actp.tile([128, 2, LQ], f32, tag="rs", name="rs")
                nc.vector.reciprocal_approx_fast(out=rs, in_=sbc)
                tmp = actp.tile([128, 2, LQ], f32, tag="tmp", name="tmp")
                nc.vector.scalar_tensor_tensor(
                    out=tmp, in0=tv[:, :, rr, :], scalar=1.0, in1=av,
                    op0=ALU.add, op1=ALU.mult)
                mt = actp.tile([128, 2, LQ], bf16, tag="mt", name="mt")
                nc.vector.scalar_tensor_tensor(
                    out=mt, in0=tmp, scalar=1.0, in1=rs, op0=ALU.mult, op1=ALU.mult)

                # ---- output projection + bo ----
                po = psp.tile([128, 2, OD], f32, tag="proj", bufs=2, name="po")
                mm_o = []
                for m in range(2):
                    for j in range(2):
                        mm_o.append(nc.tensor.matmul(
                            po[:, m, :], mt[:, j, ts(m, 128)], wo[:, j, :],
                            start=(j == 0), stop=(j == 1)))
                chain(mm_o)
                ob = actp.tile([128, 2, OD], f32, tag="ob", name="ob")
                nc.vector.scalar_tensor_tensor(
                    out=ob, in0=po, scalar=1.0, in1=bob,
                    op0=ALU.mult, op1=ALU.add)
                nc.sync.dma_start(
                    out=out_d[s].rearrange("(m p) o -> p m o", p=128), in_=ob)

            def emit_attn(st):
                sp = st["sp"]
                for rr in range(2):
                    s = 2 * sp + rr
                    et = actp.tile([128, 2, 4, 2, LQ], bf16, tag="et", name="et")
                    et2 = actp.tile([128, 2, 4, 2, LQ], bf16, tag="et2", name="et2")
                    emit_attn_L(st, rr, 0, et, et2)
                    drain_one()
                    emit_attn_L(st, rr, 1, et, et2)
                    drain_one()

                    def make_B(st=st, rr=rr, s=s, et2=et2):
                        box = {}

                        def b1():
                            sbc = psp.tile([128, 2, LQ], f32, tag="sbc", bufs=1, name="sbc")
                            av = psp.tile([128, 2, LQ], f32, tag="av", bufs=1, name="av")
                            box["t"] = (sbc, av)
                            emit_attn_B1(st, rr, et2, sbc, av)

                        def b2():
                            sbc, av = box["t"]
                            emit_attn_B2(st, rr, s, sbc, av)

                        return b1, b2

                    b1, b2 = make_B()
                    if _NO_WEAVE:
                        b1(); b2()
                    else:
                        pending.append(b1)
                        pending.append(b2)

            prev = None
            for sp in range(RPC // 2):
                st = emit_pair(sp)
                if prev is not None:
                    emit_attn(prev)
                prev = st
            emit_attn(prev)
            flush_pending()

    nc.compile()
    return nc


def _prep_inputs(q_data, kv_data, bias, nonbatched_bias, Wq, Wk, Wv, Wg, bg, Wo, bo):
    """Host-side data marshalling only (layout permutation + dtype cast)."""
    import ml_dtypes
    c = np.ascontiguousarray
    f = np.float32
    bf = ml_dtypes.bfloat16

    # [b2, lq, d] -> [b2/2, 2(c2), 128, 2(row), lq]
    def pair_layout(x):
        xt = np.swapaxes(x, 1, 2)                          # [b2, d, lq]
        xt = xt.reshape(B2 // 2, 2, 2, 128, xt.shape[-1])  # [sp, r, c2, p, l]
        return c(np.transpose(xt, (0, 2, 3, 1, 4)).astype(bf))

    qdT = pair_layout(q_data[0])
    kvdT = pair_layout(kv_data[0])

    # head dim reordered to [0,4,1,5,2,6,3,7]: position (k, half) = 2k+half so
    # psum-bank pairs (heads k, k+4) are adjacent for the wide ident adds
    horder = [4 * half + k for k in range(4) for half in range(2)]
    biasT = np.transpose(bias[0], (0, 3, 1, 2))            # [b2, lk, h, lq]
    biasT = biasT.reshape(B2, 2, 128, H, LQ)[:, :, :, horder, :]
    biasT = biasT.reshape(B2 // 2, 2, 2, 128, H, LQ).astype(bf)  # [sp,rr,c2,p,h,l]

    nbT = np.transpose(nonbatched_bias[0], (2, 0, 1))      # [lk, h, lq]
    nbT = nbT.reshape(2, 128, H, LQ)[:, :, horder, :].astype(bf)

    wqT = c(Wq.T.astype(bf))
    wkT = c(Wk.T.astype(bf))
    wvT = c(Wv.T.astype(bf))
    wgT = c(Wg.T.astype(bf))
    woT = c(Wo.T.astype(bf))
    bo_bcast = c(np.tile(np.asarray(bo, f)[None, None, :], (128, 2, 1)))
    identm = c(np.eye(128, dtype=f).astype(bf))
    two32 = np.full((128, 32), 2.0, bf)
    in_maps = []
    for core in range(NCORES):
        sl = slice(core * RPC // 2, (core + 1) * RPC // 2)
        in_maps.append(dict(
            qdT=c(qdT[sl]), kvdT=c(kvdT[sl]), biasT=c(biasT[sl]), nbT=nbT,
            wqT=wqT, wkT=wkT, wvT=wvT, wgT=wgT, woT=woT,
            bg=np.asarray(bg, f), bo_bcast=bo_bcast, ident=identm,
            two32=two32,
        ))
    return in_maps


def kernel(q_data, kv_data, bias, nonbatched_bias, Wq, Wk, Wv, Wg, bg, Wo, bo,
           _trace=False):
    from concourse.bass_utils import run_bass_kernel_spmd

    if "nc" not in _CACHE:
        _CACHE["nc"] = _build_nc()
    nc = _CACHE["nc"]
    in_maps = _prep_inputs(q_data, kv_data, bias, nonbatched_bias,
                           Wq, Wk, Wv, Wg, bg, Wo, bo)
    res = run_bass_kernel_spmd(nc, in_maps, list(range(NCORES)), trace=_trace)
    out = np.concatenate([np.asarray(res.results[i]["out"]) for i in range(NCORES)],
                         axis=0)
    out = out.reshape(B1, B2, LQ, OD).astype(np.float32, copy=False)
    if _trace:
        _CACHE["last_result"] = res
    return out



# revision 5
# speedup vs baseline: 1.0940x; 1.0940x over previous
"""AlphaFold-style gated MSA-row attention on 8 Trainium2 NeuronCores.

Shapes: q_data/kv_data [1,128,256,256], bias [1,128,8,256,256],
nonbatched_bias [1,8,256,256]; heads=8, c=32, out=256.

Strategy: pure data-parallel over b2 (128 rows -> 16 rows/core).
Per row, activations stay in "transposed" layout so softmax never needs an
on-chip transpose:
  qT/kT/gT [hc, lq]  (hc = 8*32 = 256, two 128-partition chunks)
  v        [lk, hc]
  L^T      [lk, lq]  per head = K Q^T via K=32 row-tiled matmuls (4 heads
                     run concurrently in the 32x128-tiled PE array)
  + bias^T via 128-wide identity matmuls accumulating into the psum banks
  E^T      = exp(L^T)  (ACT, PSUM->SBUF, one instr per c2 chunk)
  E2       = E * exp(nb^T)  (DVE, one instr per c2 chunk)
  S_bc     = (2*ones)^T E2  -> broadcast of 2*rowsum over 32 partitions/head
  waT      = v^T E2 (head-packed via PSUM col tiling)
  m^T      = waT * (1 + tanh((g+bg)/2)) * recip(S_bc)
  out      = m^T^T Wo^T + bo

All matmuls run bf16 inputs (full PE speed, psum accumulates f32); all large
HBM traffic (bias, nb, q, kv, weights, out) is bf16. Every DMA source/dest
is laid out p-first fully contiguous per partition so HWDGE emits one big
descriptor per partition (line-rate DMA, cheap dispatch).
Host side only re-lays-out / dtype-converts inputs - no arithmetic.
"""

import numpy as np

B1, B2, LQ, LK = 1, 128, 256, 256
QD = KVD = 256
H, C = 8, 32
HC = H * C          # 256
OD = 256
NCORES = 8
RPC = B2 // NCORES  # 16 rows per core

_CACHE = {}
_NB_DVE = True
_WARM_MMS = 14
_PREFETCH = 2      # pairs of input DMAs in flight ahead of compute


def _build_nc():
    import concourse.bass as bass
    import concourse.bacc as bacc
    import concourse.mybir as mybir
    from concourse.tile import TileContext
    from concourse.bass import ts
    from concourse.tile_rust import add_dep_helper

    f32 = mybir.dt.float32
    bf16 = mybir.dt.bfloat16
    AF = mybir.ActivationFunctionType
    ALU = mybir.AluOpType

    nc = bacc.Bacc()

    # ---- DRAM parameters (per-core shard shapes, p-first contiguous) ----
    qdT_d = nc.declare_dram_parameter("qdT", [RPC // 2, 128, 2, 2, LQ], bf16, isOutput=False)
    kvdT_d = nc.declare_dram_parameter("kvdT", [RPC // 2, 128, 2, 2, LK], bf16, isOutput=False)
    # bias packed per PAIR: [sp, 128, 2(rr), 2(c2), H, LQ]
    biasT_d = nc.declare_dram_parameter("biasT", [RPC // 2, 128, 2, 2, H, LQ], bf16, isOutput=False)
    nbT_d = nc.declare_dram_parameter("nbT", [128, 2, 4, 2, LQ], bf16, isOutput=False)
    ident_d = nc.declare_dram_parameter("ident", [128, 128], bf16, isOutput=False)
    wqT_d = nc.declare_dram_parameter("wqT", [128, 2, HC], bf16, isOutput=False)
    wkT_d = nc.declare_dram_parameter("wkT", [128, 2, HC], bf16, isOutput=False)
    wvT_d = nc.declare_dram_parameter("wvT", [128, 2, HC], bf16, isOutput=False)
    wgT_d = nc.declare_dram_parameter("wgT", [128, 2, HC], bf16, isOutput=False)
    woT_d = nc.declare_dram_parameter("woT", [128, 2, OD], bf16, isOutput=False)
    bg_d = nc.declare_dram_parameter("bg", [128, 2], f32, isOutput=False)
    bob_d = nc.declare_dram_parameter("bo_bcast", [128, 2, OD], f32, isOutput=False)
    two32_d = nc.declare_dram_parameter("two32", [128, 32], bf16, isOutput=False)
    out_d = nc.declare_dram_parameter("out", [RPC, 128, 2, OD], bf16, isOutput=True)

    def chain(mms):
        for a, b in zip(mms, mms[1:]):
            add_dep_helper(b.ins, a.ins, sync=False, reason="psum bank group order")

    with TileContext(nc) as tc:
        with tc.tile_pool(name="const", bufs=1) as constp, \
             tc.tile_pool(name="io", bufs=3) as iop, \
             tc.tile_pool(name="act", bufs=2) as actp, \
             tc.tile_pool(name="ps", bufs=1, space="PSUM") as psp:

            # ---- one-time loads (order matters: everything small lands
            # before the first 2MB bias transfer can queue behind it) ----
            ident = constp.tile([128, 128], bf16, name="ident")
            wq = constp.tile([128, 2, HC], bf16, name="wq")
            wk = constp.tile([128, 2, HC], bf16, name="wk")
            wv = constp.tile([128, 2, HC], bf16, name="wv")
            wg = constp.tile([128, 2, HC], bf16, name="wg")
            wo = constp.tile([128, 2, OD], bf16, name="wo")
            nbt = constp.tile([128, 2, 4, 2, LQ], bf16, name="nbt")
            enb = constp.tile([128, 2, 4, 2, LQ], bf16, name="enb")
            two32 = constp.tile([128, 32], bf16, name="two32")
            bob = constp.tile([128, 2, OD], f32, name="bob")
            bg_sb = constp.tile([128, 2], f32, name="bg_sb")
            bgh = constp.tile([128, 2], f32, name="bgh")

            nc.sync.dma_start(out=ident, in_=ident_d[:, :])
            for t, d in ((wq, wqT_d), (wk, wkT_d), (wv, wvT_d), (wg, wgT_d)):
                nc.sync.dma_start(out=t, in_=d[:, :, :])

            # ---- PE warm-up: dummy matmuls so the HAM clock gate reaches
            # 8/8 before the first real projection. Reads the (landed)
            # ident/weight tiles; reuses the sbc psum bank (psum is full).
            pwarm = psp.tile([128, 2, LQ], f32, tag="sbc", bufs=1, name="pwarm")
            wmm = []
            for i in range(_WARM_MMS):
                wmm.append(nc.tensor.matmul(
                    pwarm[:, :, :], ident, wq[:, :, :],
                    start=True, stop=True, skip_group_check=True))
            chain(wmm)

            def load_consts():
                nc.sync.dma_start(out=two32, in_=two32_d[:, :])
                nc.sync.dma_start(out=nbt, in_=nbT_d[:, :, :, :, :])
                nc.scalar.activation(enb, nbt, AF.Exp)
                nc.sync.dma_start(out=bg_sb, in_=bg_d[:, :])
                nc.scalar.mul(bgh, bg_sb, 0.5)
                nc.sync.dma_start(out=bob, in_=bob_d[:, :, :])
                nc.sync.dma_start(out=wo, in_=woT_d[:, :, :])

            qscale = float(C) ** -0.5

            def issue_dmas(sp):
                qdt = iop.tile([128, 2, 2, LQ], bf16, tag="qdt", name="qdt")
                kvdt = iop.tile([128, 2, 2, LK], bf16, tag="kvdt", name="kvdt")
                nc.sync.dma_start(out=qdt, in_=qdT_d[sp])
                nc.sync.dma_start(out=kvdt, in_=kvdT_d[sp])
                if sp == 0:
                    load_consts()
                bias_sb = iop.tile([128, 2, 2, H, LQ], bf16, tag="bias", bufs=4, name="bias_sb")
                nc.sync.dma_start(out=bias_sb, in_=biasT_d[sp])
                return dict(sp=sp, qdt=qdt, kvdt=kvdt, bias_sb=bias_sb)

            def emit_pair(st):
                qdt, kvdt = st["qdt"], st["kvdt"]
                # ---- paired projections q/k/g (N=512 over 2 rows) ----
                pq_j = [psp.tile([128, 2, LQ], f32, tag="proj", bufs=2, name=f"pq{j}") for j in range(2)]
                pk_j = [psp.tile([128, 2, LK], f32, tag="proj", bufs=2, name=f"pk{j}") for j in range(2)]
                pg_j = [psp.tile([128, 2, LQ], f32, tag="proj", bufs=2, name=f"pg{j}") for j in range(2)]
                for pX, w, rhs_t in ((pq_j, wq, qdt), (pk_j, wk, kvdt), (pg_j, wg, qdt)):
                    for j in range(2):
                        mms = []
                        for c2 in range(2):
                            mms.append(nc.tensor.matmul(
                                pX[j][:, :, :], w[:, c2, ts(j, 128)], rhs_t[:, c2, :, :],
                                start=(c2 == 0), stop=(c2 == 1)))
                        chain(mms)

                sq = actp.tile([128, 2, 2, LQ], bf16, tag="sq", name="sq")
                sk = actp.tile([128, 2, 2, LK], bf16, tag="sk", name="sk")
                tv = actp.tile([128, 2, 2, LQ], f32, tag="tv", name="tv")
                for j in range(2):
                    nc.vector.tensor_scalar_mul(sq[:, j, :, :], pq_j[j], qscale)
                    nc.vector.tensor_copy(sk[:, j, :, :], pk_j[j])
                    nc.scalar.activation(tv[:, j, :, :], pg_j[j], AF.Tanh,
                                         bias=bgh[:, ts(j, 1)], scale=0.5)

                sv_r = []
                for rr in range(2):
                    # ---- v projection (per row) ----
                    pv = psp.tile([128, 2, HC], f32, tag="proj", bufs=2, name="pv")
                    mm_v = []
                    for j in range(2):
                        for c2 in range(2):
                            mm_v.append(nc.tensor.matmul(
                                pv[:, j, :], kvdt[:, c2, rr, ts(j, 128)], wv[:, c2, :],
                                start=(c2 == 0), stop=(c2 == 1)))
                    chain(mm_v)
                    sv = actp.tile([128, 2, HC], bf16, tag="sv", bufs=4, name="sv")
                    nc.vector.tensor_copy(sv, pv)
                    sv_r.append(sv)
                st.update(sq=sq, sk=sk, tv=tv, sv_r=sv_r)
                return st

            # Software pipeline: the sbc/av ("B") PE work of row r-1 is emitted
            # BETWEEN the logits/exp ("L") chunks of row r, so the PE never
            # idles waiting for exp (pL is single-buffered).
            pending = []

            def drain_one():
                if pending:
                    pending.pop(0)()

            def flush_pending():
                while pending:
                    pending.pop(0)()

            def emit_attn_L(st, rr, c2, et, et2):
                sq, sk, bias_sb = st["sq"], st["sk"], st["bias_sb"]
                pL = psp.tile([128, 4, 2, LQ], f32, tag="L", bufs=1, name="pL")
                # start=True resets the WHOLE psum bank -> exactly one start
                # per bank (head h, half=0); head h+4 (same bank, other column
                # half) accumulates onto the zeroed bank.
                grp = [[] for _ in range(4)]
                for h in range(H):
                    k, half = h % 4, h // 4
                    grp[k].append(nc.tensor.matmul(
                        pL[:, k, half, :],
                        sk[ts(k, 32), h // 4, rr, ts(c2, 128)],
                        sq[ts(k, 32), h // 4, rr, :],
                        start=(half == 0), stop=False, skip_group_check=True,
                        tile_position=(32 * k, 0)))
                # bias adds: wide full-identity matmuls per bank
                for k in range(4):
                    grp[k].append(nc.tensor.matmul(
                        pL[:, k, :, :], ident,
                        bias_sb[:, rr, c2, ts(k, 2), :],
                        start=False, stop=True, skip_group_check=True))
                for k in range(4):
                    chain(grp[k])
                # exp of the whole c2 chunk in ONE ACT instr (N=2048)
                nc.scalar.activation(et[:, c2, :, :, :], pL[:, :, :, :], AF.Exp)
                # nb applied multiplicatively: E2 = E * exp(nb^T), one DVE
                # instr (bf16 sbuf->sbuf 2x mode)
                nc.vector.tensor_mul(et2[:, c2, :, :, :], et[:, c2, :, :, :],
                                     enb[:, c2, :, :, :])

            def emit_attn_B1(st, rr, et, sbc, av):
                # sbc/av: full contraction over lk. sbc merges both g groups
                # into N=512 matmuls (same two32 stationary); av keeps
                # per-head stationaries (N=256).
                sv = st["sv_r"][rr]
                mm_s, mm_a = [], []
                for c2 in range(2):
                    for hh in range(4):
                        mm_s.append(nc.tensor.matmul(
                            sbc[ts(hh, 32), :, :], two32, et[:, c2, hh, :, :],
                            start=(c2 == 0), stop=(c2 == 1),
                            tile_position=(0, 32 * hh), skip_group_check=True))
                for g in range(2):
                    for c2 in range(2):
                        for hh in range(4):
                            h = 4 * g + hh
                            eth = et[:, c2, h % 4, h // 4, :]
                            mm_a.append(nc.tensor.matmul(
                                av[ts(hh, 32), g, :], sv[:, c2, ts(h, 32)], eth,
                                start=(c2 == 0), stop=(c2 == 1),
                                tile_position=(0, 32 * hh), skip_group_check=True))
                chain(mm_s)
                chain(mm_a)
                return None

            def emit_attn_B2(st, rr, s, sbc, av):
                tv = st["tv"]
                rs = actp.tile([128, 2, LQ], f32, tag="rs", name="rs")
                nc.vector.reciprocal_approx_fast(out=rs, in_=sbc)
                tmp = actp.tile([128, 2, LQ], f32, tag="tmp", name="tmp")
                nc.vector.scalar_tensor_tensor(
                    out=tmp, in0=tv[:, :, rr, :], scalar=1.0, in1=av,
                    op0=ALU.add, op1=ALU.mult)
                mt = actp.tile([128, 2, LQ], bf16, tag="mt", name="mt")
                nc.vector.scalar_tensor_tensor(
                    out=mt, in0=tmp, scalar=1.0, in1=rs, op0=ALU.mult, op1=ALU.mult)

                # ---- output projection + bo ----
                po = psp.tile([128, 2, OD], f32, tag="proj", bufs=2, name="po")
                mm_o = []
                for m in range(2):
                    for j in range(2):
                        mm_o.append(nc.tensor.matmul(
                            po[:, m, :], mt[:, j, ts(m, 128)], wo[:, j, :],
                            start=(j == 0), stop=(j == 1)))
                chain(mm_o)
                ob = actp.tile([128, 2, OD], bf16, tag="ob", name="ob")
                nc.vector.scalar_tensor_tensor(
                    out=ob, in0=po, scalar=1.0, in1=bob,
                    op0=ALU.mult, op1=ALU.add)
                nc.sync.dma_start(out=out_d[s], in_=ob)

            def emit_attn(st):
                sp = st["sp"]
                for rr in range(2):
                    s = 2 * sp + rr
                    et = actp.tile([128, 2, 4, 2, LQ], bf16, tag="et", name="et")
                    et2 = actp.tile([128, 2, 4, 2, LQ], bf16, tag="et2", name="et2")
                    emit_attn_L(st, rr, 0, et, et2)
                    drain_one()
                    emit_attn_L(st, rr, 1, et, et2)
                    drain_one()

                    def make_B(st=st, rr=rr, s=s, et2=et2):
                        box = {}

                        def b1():
                            sbc = psp.tile([128, 2, LQ], f32, tag="sbc", bufs=1, name="sbc")
                            av = psp.tile([128, 2, LQ], f32, tag="av", bufs=1, name="av")
                            box["t"] = (sbc, av)
                            emit_attn_B1(st, rr, et2, sbc, av)

                        def b2():
                            sbc, av = box["t"]
                            emit_attn_B2(st, rr, s, sbc, av)

                        return b1, b2

                    b1, b2 = make_B()
                    pending.append(b1)
                    pending.append(b2)

            NP = RPC // 2
            sts = {}
            for sp in range(min(_PREFETCH, NP)):
                sts[sp] = issue_dmas(sp)
            prev = None
            for sp in range(NP):
                st = emit_pair(sts.pop(sp))
                if sp + _PREFETCH < NP:
                    sts[sp + _PREFETCH] = issue_dmas(sp + _PREFETCH)
                if prev is not None:
                    emit_attn(prev)
                prev = st
            emit_attn(prev)
            flush_pending()

    nc.compile()
    return nc


def _prep_inputs(q_data, kv_data, bias, nonbatched_bias, Wq, Wk, Wv, Wg, bg, Wo, bo):
    """Host-side data marshalling only (layout permutation + dtype cast).

    Every DRAM tensor is laid out p-first with all remaining dims contiguous
    per partition, so each dma_start is a single linear run per partition.
    """
    import ml_dtypes
    c = np.ascontiguousarray
    f = np.float32
    bf = ml_dtypes.bfloat16

    # [b2, l, d] -> [sp, p, c2, rr, l] (d = 128*c2 + p)
    def pair_layout(x):
        xt = x.reshape(B2 // 2, 2, x.shape[1], 2, 128)     # [sp, rr, l, c2, p]
        return c(np.transpose(xt, (0, 4, 3, 1, 2)).astype(bf))

    qdT = pair_layout(q_data[0])
    kvdT = pair_layout(kv_data[0])

    # head dim reordered to [0,4,1,5,2,6,3,7]: position (k, half) = 2k+half so
    # psum-bank pairs (heads k, k+4) are adjacent for the wide ident adds
    horder = [4 * half + k for k in range(4) for half in range(2)]
    # bias [b2, h, lq, lk] -> [sp, p, rr, c2, h', lq] (lk = 128*c2 + p)
    biasT = bias[0][:, horder]                              # [b2, h', lq, lk]
    biasT = biasT.reshape(B2 // 2, 2, H, LQ, 2, 128)        # [sp, rr, h, lq, c2, p]
    biasT = c(np.transpose(biasT, (0, 5, 1, 4, 2, 3)).astype(bf))

    # nb [h, lq, lk] -> [p, c2, k, half, lq] with h = 4*half + k
    nbT = nonbatched_bias[0].reshape(2, 4, LQ, 2, 128)      # [half, k, lq, c2, p]
    nbT = c(np.transpose(nbT, (4, 3, 1, 0, 2)).astype(bf))

    def w_layout(w):  # [out=hc, in=d] -> [p, c2, hc]  (d = 128*c2 + p)
        return c(np.transpose(w.T.reshape(2, 128, HC), (1, 0, 2)).astype(bf))

    wqT = w_layout(Wq)
    wkT = w_layout(Wk)
    wvT = w_layout(Wv)
    wgT = w_layout(Wg)
    woT = c(np.transpose(np.asarray(Wo).T.reshape(2, 128, OD), (1, 0, 2)).astype(bf))
    bgp = c(np.asarray(bg, f).reshape(2, 128).T)
    bo_bcast = c(np.tile(np.asarray(bo, f)[None, None, :], (128, 2, 1)))
    identm = c(np.eye(128, dtype=f).astype(bf))
    two32 = np.full((128, 32), 2.0, bf)
    in_maps = []
    for core in range(NCORES):
        sl = slice(core * RPC // 2, (core + 1) * RPC // 2)
        in_maps.append(dict(
            qdT=c(qdT[sl]), kvdT=c(kvdT[sl]), biasT=c(biasT[sl]), nbT=nbT,
            wqT=wqT, wkT=wkT, wvT=wvT, wgT=wgT, woT=woT,
            bg=bgp, bo_bcast=bo_bcast, ident=identm,
            two32=two32,
        ))
    return in_maps


def kernel(q_data, kv_data, bias, nonbatched_bias, Wq, Wk, Wv, Wg, bg, Wo, bo,
           _trace=False):
    from concourse.bass_utils import run_bass_kernel_spmd

    if "nc" not in _CACHE:
        _CACHE["nc"] = _build_nc()
    nc = _CACHE["nc"]
    in_maps = _prep_inputs(q_data, kv_data, bias, nonbatched_bias,
                           Wq, Wk, Wv, Wg, bg, Wo, bo)
    res = run_bass_kernel_spmd(nc, in_maps, list(range(NCORES)), trace=_trace)
    # out_d is [RPC, 128, 2, OD] bf16 with lq = 128*m + p
    outs = []
    for i in range(NCORES):
        o = np.asarray(res.results[i]["out"]).astype(np.float32)
        o = np.transpose(o, (0, 2, 1, 3)).reshape(RPC, LQ, OD)
        outs.append(o)
    out = np.concatenate(outs, axis=0).reshape(B1, B2, LQ, OD)
    if _trace:
        _CACHE["last_result"] = res
    return out
